# revision 1
# baseline (speedup 1.0000x reference)
"""ClusterNorm1d TRN2 kernel.

Math (per cluster k): mu = mean_b x[b,:,k]; cov = centered second moment;
L = chol(cov + eps I); Z = L^-1 (x - mu).  Output Z transposed back.

Strategy per core (32 clusters): K-sharded across 8 cores, no collectives.
  - stats: bf16 matmuls over a host-prepared [b, (d|1)] tensor, accumulating
    U^T U = [[S, s], [s^T, B]] in fp32 PSUM (32 accumulating matmuls).
  - cov -> W = L^-1 via 4 Newton iterations on the Cholesky manifold:
      P = W A W^T;  C^T = CM o (I - P);  W <- W + C^T^T W
    (CM = triu(1) + 0.5 I).  Converges quadratically; exact-fp32 validated.
  - solve: Z = W x - (W mu) 1^T as float32r matmuls (1 cyc/row @ N=512),
    mean applied as per-partition bias during the PSUM->SBUF copy.
Host supplies x pre-transposed per core as [32, 64, 4096] (f32r) and the
bf16 stats operand [32, 4096, 66] (col 64 = ones, col 65 pad).
"""
import sys
sys.path.insert(0, "/opt/trn_rl_repo")

import numpy as np
import ml_dtypes

import concourse.bass as bass
from concourse import bacc
import concourse.mybir as mybir
import concourse.tile as tile
from concourse.bass_utils import run_bass_kernel_spmd

B, D, K, NCORES = 4096, 64, 256, 8
KL = K // NCORES          # clusters per core
EPS = 1e-4
NB = B // 512             # solve chunks per cluster
AF = mybir.ActivationFunctionType

_cache = {}


def _build_nc(repeat=1):
    nc = bacc.Bacc("TRN2", target_bir_lowering=False, debug=False,
                   num_devices=NCORES)
    d_xs = nc.dram_tensor("xs", [KL, D, B], mybir.dt.float32r,
                          kind="ExternalInput")
    d_xb = nc.dram_tensor("xb", [KL, B, 66], mybir.dt.bfloat16,
                          kind="ExternalInput")
    d_cs = nc.dram_tensor("cs", [D, 4 * D], mybir.dt.float32,
                          kind="ExternalInput")
    d_out = nc.dram_tensor("out", [KL, D, B], mybir.dt.float32,
                           kind="ExternalOutput")

    inv_b = 1.0 / B
    a_cov = 1.0 / (B - 1)
    b_cov = 1.0 / (B * (B - 1.0))

    with tile.TileContext(nc) as tc:
        with tc.tile_pool(name="consts", bufs=1) as consts, \
             tc.tile_pool(name="slab", bufs=2) as slabp, \
             tc.tile_pool(name="upool", bufs=2) as upool, \
             tc.tile_pool(name="zpool", bufs=2) as zpool, \
             tc.tile_pool(name="small", bufs=4) as small, \
             tc.tile_pool(name="wpool", bufs=8) as wpool, \
             tc.tile_pool(name="ps_stat", bufs=2, space="PSUM") as ps_stat, \
             tc.tile_pool(name="ps_small", bufs=4, space="PSUM") as ps_small, \
             tc.tile_pool(name="ps_z", bufs=2, space="PSUM") as ps_z:

            tcs = consts.tile([D, 4 * D], mybir.dt.float32)
            nc.sync.dma_start(out=tcs, in_=d_cs.ap())
            ident = tcs[:, 0:D]
            cmask = tcs[:, D:2 * D]        # triu(1,k=1) + 0.5 I
            chalf = tcs[:, 2 * D:3 * D]    # 0.5 I
            epsi = tcs[:, 3 * D:4 * D]     # EPS * I

            for p0 in range(repeat * (KL // 2)):
                p = p0 % (KL // 2)
                k0, k1 = 2 * p, 2 * p + 1
                # ---- x slab for the pair: [128, 4096] f32r, full-width ----
                slab = slabp.tile([2 * D, B], mybir.dt.float32r)
                nc.sync.dma_start(
                    out=slab,
                    in_=d_xs.ap()[k0:k0 + 2].rearrange("c d b -> (c d) b"))

                zpair = zpool.tile([2 * D, B], mybir.dt.float32)
                outdma_deps = []

                for half, kk in enumerate((k0, k1)):
                    # ---- stats ----
                    ub = upool.tile([128, (B // 128) * 66], mybir.dt.bfloat16)
                    nc.scalar.dma_start(
                        out=ub,
                        in_=d_xb.ap()[kk].rearrange("(p j) c -> p (j c)",
                                                    p=128))
                    ps = ps_stat.tile([D + 1, D + 1], mybir.dt.float32)
                    for j in range(B // 128):
                        sl = ub[:, 66 * j:66 * j + 65]
                        nc.tensor.matmul(ps, sl, sl, start=(j == 0),
                                         stop=(j == B // 128 - 1))
                    st = small.tile([D + 1, D + 1], mybir.dt.float32,
                                    tag="st")
                    nc.scalar.copy(st, ps)

                    # ---- s s^T via K=2 matmul at base 0 ----
                    z2 = small.tile([2, D + 1], mybir.dt.float32, tag="z2")
                    nc.vector.memset(z2, 0.0)
                    nc.scalar.copy(z2[0:1, :], st[D:D + 1, :])
                    pso = ps_small.tile([D, D], mybir.dt.float32, tag="ps64")
                    nc.tensor.matmul(pso, z2[:, 0:D], z2[:, 0:D],
                                     start=True, stop=True)

                    # ---- cov A = S/(B-1) - s s^T/(B(B-1)) + eps I ----
                    t1 = small.tile([D, D], mybir.dt.float32, tag="t1")
                    nc.vector.tensor_scalar_mul(t1, st[0:D, 0:D], a_cov)
                    t2 = small.tile([D, D], mybir.dt.float32, tag="t2")
                    nc.vector.tensor_scalar_mul(t2, pso, b_cov)
                    t3 = small.tile([D, D], mybir.dt.float32, tag="t3")
                    nc.vector.tensor_sub(t3, t1, t2)
                    amat = small.tile([D, D], mybir.dt.float32, tag="amat")
                    nc.vector.tensor_add(amat, t3, epsi)

                    # ---- Newton with W0 = I (A ~ I): it0 analytic ----
                    u1 = small.tile([D, D], mybir.dt.float32, tag="u1")
                    nc.vector.tensor_mul(u1, cmask, amat)
                    ct = small.tile([D, D], mybir.dt.float32, tag="ct")
                    nc.vector.tensor_sub(ct, chalf, u1)
                    psd = ps_small.tile([D, D], mybir.dt.float32, tag="ps64")
                    nc.tensor.matmul(psd, ct, ident, start=True, stop=True)
                    w = wpool.tile([D, D], mybir.dt.float32, tag="w")
                    nc.vector.tensor_add(w, ident, psd)
                    NIT = 4
                    for it in range(1, NIT):
                        pst = ps_small.tile([D, D], mybir.dt.float32,
                                            tag="ps64")
                        nc.tensor.transpose(pst, w, ident)
                        wt = wpool.tile([D, D], mybir.dt.float32, tag="wt")
                        nc.scalar.copy(wt, pst)
                        psh = ps_small.tile([D, D], mybir.dt.float32,
                                            tag="ps64")
                        nc.tensor.matmul(psh, amat, wt, start=True, stop=True)
                        h = small.tile([D, D], mybir.dt.float32, tag="h")
                        nc.scalar.copy(h, psh)
                        psp = ps_small.tile([D, D], mybir.dt.float32,
                                            tag="ps64")
                        nc.tensor.matmul(psp, wt, h, start=True, stop=True)
                        u1 = small.tile([D, D], mybir.dt.float32, tag="u1")
                        nc.vector.tensor_mul(u1, cmask, psp)
                        ct = small.tile([D, D], mybir.dt.float32, tag="ct")
                        nc.vector.tensor_sub(ct, chalf, u1)
                        psd = ps_small.tile([D, D], mybir.dt.float32,
                                            tag="ps64")
                        nc.tensor.matmul(psd, ct, w, start=True, stop=True)
                        wn = wpool.tile([D, D], mybir.dt.float32, tag="w")
                        nc.vector.tensor_add(wn, w, psd)
                        w = wn

                    # ---- final W^T as stacked f32r solve weights [128,64]:
                    #      own half = W^T, other half = 0 (K=128 matmul) ----
                    pst = ps_small.tile([D, D], mybir.dt.float32, tag="ps64")
                    nc.tensor.transpose(pst, w, ident)
                    wtr = wpool.tile([2 * D, D], mybir.dt.float32r, tag="wtr")
                    nc.scalar.copy(wtr[half * D:(half + 1) * D, :], pst)
                    nc.scalar.activation(
                        out=wtr[(1 - half) * D:(2 - half) * D, :],
                        in_=slab[0:D, 0:D], func=AF.Identity, scale=0.0)

                    # ---- v = W mu; bias = -v ----
                    mur = small.tile([2 * D, 2], mybir.dt.float32r, tag="mur")
                    nc.scalar.activation(out=mur, in_=slab[:, 0:2],
                                         func=AF.Identity, scale=0.0)
                    nc.scalar.activation(out=mur[half * D:(half + 1) * D, 0:1],
                                         in_=st[0:D, D:D + 1],
                                         func=AF.Identity, scale=inv_b)
                    psv = ps_small.tile([D, 2], mybir.dt.float32, tag="ps64")
                    nc.tensor.matmul(psv, wtr, mur, start=True, stop=True)
                    biask = small.tile([D, 1], mybir.dt.float32, tag="biask")
                    nc.scalar.activation(out=biask, in_=psv[:, 0:1],
                                         func=AF.Identity, scale=-1.0)

                    # ---- solve: Z = W x + bias ----
                    for j in range(NB):
                        psz = ps_z.tile([D, 512], mybir.dt.float32, tag="psz")
                        nc.tensor.matmul(
                            psz, wtr,
                            slab[:, 512 * j: 512 * (j + 1)],
                            start=True, stop=True)
                        dst = zpair[half * D:(half + 1) * D,
                                    512 * j:512 * (j + 1)]
                        if half == 0:
                            cp = nc.scalar.activation(out=dst, in_=psz,
                                                      func=AF.Identity,
                                                      bias=biask)
                        else:
                            cp = nc.vector.tensor_scalar_add(dst, psz, biask)
                        outdma_deps.append(cp)

                nc.sync.dma_start(
                    out=d_out.ap()[k0:k0 + 2].rearrange("c d b -> (c d) b"),
                    in_=zpair)

    nc.finalize()
    return nc


def _make_consts():
    ident = np.eye(D, dtype=np.float32)
    cmask = np.triu(np.ones((D, D), np.float32), 1) + 0.5 * ident
    chalf = 0.5 * ident
    epsi = EPS * ident
    return np.concatenate([ident, cmask, chalf, epsi], axis=1)


def _prep_inputs(x):
    """x: [B, D, K] fp32 -> per-core input dicts."""
    consts = _make_consts()
    in_maps = []
    for c in range(NCORES):
        ks = slice(c * KL, (c + 1) * KL)
        xs = np.ascontiguousarray(x[:, :, ks].transpose(2, 1, 0))  # [KL, D, B]
        xt = xs.transpose(0, 2, 1)                                  # [KL, B, D]
        xb = np.empty((KL, B, 66), dtype=ml_dtypes.bfloat16)
        xb[:, :, 0:D] = xt.astype(ml_dtypes.bfloat16)
        xb[:, :, D] = np.float32(1.0)
        xb[:, :, D + 1] = np.float32(0.0)
        in_maps.append({"xs": xs, "xb": xb, "cs": consts})
    return in_maps


def _run(x, trace=False):
    if "nc" not in _cache:
        _cache["nc"] = _build_nc()
    nc = _cache["nc"]
    in_maps = _prep_inputs(np.asarray(x, dtype=np.float32))
    res = run_bass_kernel_spmd(nc, in_maps, core_ids=list(range(NCORES)),
                               trace=trace)
    out = np.empty((B, D, K), dtype=np.float32)
    for c in range(NCORES):
        ks = slice(c * KL, (c + 1) * KL)
        out[:, :, ks] = res.results[c]["out"].transpose(2, 1, 0)
    return out, res


def kernel(x):
    out, _ = _run(x, trace=False)
    return out



# revision 6
# speedup vs baseline: 2.8561x; 2.8561x over previous
"""ClusterNorm1d TRN2 kernel (pair-batched, low-precision, phase-pipelined).

Math (per cluster k): mu = mean_b x[b,:,k]; cov = centered second moment;
L = chol(cov + eps I); Z = L^-1 (x - mu).

Per core: 32 clusters processed as 16 PAIRS (two 64-dim clusters batched
block-diagonally into 128-wide PE ops).

  - stats: per pair, 32 accumulating bf16 matmuls G += U_j^T [U_j | 1]
    ([128,129] out in fp32 PSUM).  Diag 64x64 blocks of G[:, :128] are the
    two clusters' Grams; col 128 is the per-column sum s (mean * B).
  - cov -> W = L^-1 via Newton on the Cholesky manifold, all in fp16
    (1 cyc/row on PE vs 4 for fp32; iteration is self-correcting):
      P = W A W^T;  I + C^T = 1.5 I - CM o P;  W <- (I+C) W
    keeping both W and W^T via two matmuls per step (no transposes).
  - solve: Z = W x - (W mu) 1^T as bf16 matmuls, mean applied as
    per-partition bias during the PSUM -> SBUF copy; output stored bf16.
Inputs shipped bf16 twice (b-major for stats, d-major for solve), output
bf16: ~51 MB HBM traffic per core vs 85 MB for the f32 layout.
"""
import sys
sys.path.insert(0, "/opt/trn_rl_repo")

import numpy as np
import ml_dtypes

import concourse.bass as bass
from concourse import bacc
import concourse.mybir as mybir
import concourse.tile as tile
from concourse.bass_utils import run_bass_kernel_spmd

B, D, K, NCORES = 4096, 64, 256, 8
KL = K // NCORES          # clusters per core (32)
NP = KL // 2              # cluster pairs per core (16)
NCH = B // 128            # stats chunks (32)
NB = B // 512             # solve chunks per pair (8)
EPS = 1e-4
NIT = 4                   # newton steps incl analytic it0
AF = mybir.ActivationFunctionType
F32 = mybir.dt.float32
F16 = mybir.dt.float16
BF16 = mybir.dt.bfloat16

_cache = {}


def _build_nc():
    nc = bacc.Bacc("TRN2", target_bir_lowering=False, debug=False,
                   num_devices=NCORES)
    d_xb = nc.dram_tensor("xb", [NP, 128, NCH * 130], BF16,
                          kind="ExternalInput")
    d_xs = nc.dram_tensor("xs", [NP, 128, B], BF16, kind="ExternalInput")
    d_cs = nc.dram_tensor("cs", [128, 5 * 128], F32, kind="ExternalInput")
    d_id = nc.dram_tensor("id16", [128, 128], F16, kind="ExternalInput")
    d_out = nc.dram_tensor("out", [NP, 128, B], BF16, kind="ExternalOutput")

    inv_b = 1.0 / B
    a_cov = 1.0 / (B - 1)
    sq_bcov = float(np.sqrt(1.0 / (B * (B - 1.0))))

    with tile.TileContext(nc) as tc:
        with tc.tile_pool(name="consts", bufs=1) as consts, \
             tc.tile_pool(name="ubp", bufs=3) as ubp, \
             tc.tile_pool(name="slabp", bufs=4) as slabp, \
             tc.tile_pool(name="zp", bufs=2) as zp, \
             tc.tile_pool(name="smf", bufs=3) as smf, \
             tc.tile_pool(name="s16", bufs=3) as s16, \
             tc.tile_pool(name="wp", bufs=2 * NP + 4) as wp, \
             tc.tile_pool(name="pp", bufs=NP) as pp, \
             tc.tile_pool(name="ps_big", bufs=2, space="PSUM") as ps_big, \
             tc.tile_pool(name="ps_sm", bufs=4, space="PSUM") as ps_sm, \
             tc.tile_pool(name="ps_row", bufs=1, space="PSUM") as ps_row:

            cs = consts.tile([128, 5 * 128], F32)
            nc.sync.dma_start(out=cs, in_=d_cs.ap())
            cmask = cs[:, 0:128]          # blkdiag(triu(64,1) + 0.5 I)
            blkm = cs[:, 128:256]         # blkdiag(ones 64x64)
            epsi = cs[:, 256:384]         # EPS * I
            c15 = cs[:, 384:512]          # 1.5 * I
            c15e = cs[:, 512:640]         # (1.5 - 0.5*EPS) * I
            id16 = consts.tile([128, 128], F16)
            nc.sync.dma_start(out=id16, in_=d_id.ap())

            P = [dict() for _ in range(NP)]

            # ---------------- Phase A: stats + cov prep ----------------
            def emit_prep(p):
                psg = P[p]["psg"]
                srow = s16.tile([128, 1], F16, tag="srow")
                nc.scalar.activation(out=srow, in_=psg[:, 128:129],
                                     func=AF.Identity, scale=sq_bcov)
                mur = pp.tile([128, 1], BF16, tag="mur")
                nc.scalar.activation(out=mur, in_=psg[:, 128:129],
                                     func=AF.Identity, scale=inv_b)
                pst = ps_row.tile([1, 128], F16, tag="pst")
                nc.tensor.transpose(pst, srow, id16)
                z2 = s16.tile([1, 128], F16, tag="z2")
                nc.scalar.copy(z2, pst)
                pso = ps_sm.tile([128, 128], F32, tag="sm")
                nc.tensor.matmul(pso, z2, z2, start=True, stop=True)
                t1 = smf.tile([128, 128], F32, tag="t1")
                nc.vector.tensor_scalar_mul(t1, psg[:, 0:128], a_cov)
                t3 = smf.tile([128, 128], F32, tag="t3")
                nc.vector.tensor_sub(t3, t1, pso)
                u1 = smf.tile([128, 128], F32, tag="u1")
                nc.vector.tensor_mul(u1, cmask, t3)
                w1t = wp.tile([128, 128], F16, tag="wt")
                nc.vector.tensor_sub(w1t, c15e, u1)
                am = smf.tile([128, 128], F32, tag="am")
                nc.vector.tensor_mul(am, blkm, t3)
                amat = pp.tile([128, 128], F16, tag="amat")
                nc.vector.tensor_add(amat, am, epsi)
                P[p].update(wt=w1t, amat=amat, mur=mur)

            for p in range(NP):
                ub = ubp.tile([128, NCH * 130], BF16, tag="ub")
                nc.scalar.dma_start(out=ub, in_=d_xb.ap()[p])
                psg = ps_big.tile([128, 512], F32, tag="big")
                for j in range(NCH):
                    nc.tensor.matmul(psg[:, 0:129],
                                     ub[:, 130 * j:130 * j + 128],
                                     ub[:, 130 * j:130 * j + 129],
                                     start=(j == 0), stop=(j == NCH - 1))
                P[p]["psg"] = psg
                if p >= 1:
                    emit_prep(p - 1)
            emit_prep(NP - 1)

            # ---------------- Phase B: Newton (step-interleaved) -------
            # w1 = (w1t)^T via matmul against identity
            for p in range(NP):
                psw1 = ps_sm.tile([128, 128], F32, tag="sm")
                nc.tensor.matmul(psw1, P[p]["wt"], id16, start=True,
                                 stop=True)
                w = wp.tile([128, 128], F16, tag="w")
                nc.scalar.copy(w, psw1)
                P[p]["w"] = w

            for it in range(1, NIT):
                last = (it == NIT - 1)
                for p in range(NP):
                    psh = ps_sm.tile([128, 128], F32, tag="sm")
                    nc.tensor.matmul(psh, P[p]["amat"], P[p]["wt"],
                                     start=True, stop=True)
                    h = s16.tile([128, 128], F16, tag="h", bufs=NP + 2)
                    nc.scalar.copy(h, psh)
                    P[p]["h"] = h
                for p in range(NP):
                    psp = ps_sm.tile([128, 128], F32, tag="sm")
                    nc.tensor.matmul(psp, P[p]["wt"], P[p]["h"],
                                     start=True, stop=True)
                    u1 = smf.tile([128, 128], F32, tag="u1")
                    nc.vector.tensor_mul(u1, cmask, psp)
                    ctI = s16.tile([128, 128], F16, tag="ct", bufs=NP + 2)
                    nc.vector.tensor_sub(ctI, c15, u1)
                    P[p]["ctI"] = ctI
                for p in range(NP):
                    pswt = ps_sm.tile([128, 128], F32, tag="sm")
                    nc.tensor.matmul(pswt, P[p]["w"], P[p]["ctI"],
                                     start=True, stop=True)
                    if last:
                        wtn = wp.tile([128, 128], BF16, tag="wtb")
                        nc.scalar.copy(wtn, pswt)
                        P[p]["wt"] = wtn
                    else:
                        wtn = wp.tile([128, 128], F16, tag="wt")
                        nc.scalar.copy(wtn, pswt)
                        P[p]["wt"] = wtn
                        psw = ps_sm.tile([128, 128], F32, tag="sm")
                        nc.tensor.matmul(psw, P[p]["ctI"], P[p]["w"],
                                         start=True, stop=True)
                        wn = wp.tile([128, 128], F16, tag="w")
                        nc.scalar.copy(wn, psw)
                        P[p]["w"] = wn

            # bias vector v = W mu (bf16 matmul, N=1) in its own row so
            # psv(p) never waits on the just-written wt copy
            for p in range(NP):
                psv = ps_row.tile([128, 2], F32, tag="psv")
                nc.tensor.matmul(psv[:, 0:1], P[p]["wt"], P[p]["mur"],
                                 start=True, stop=True)
                biask = pp.tile([128, 1], F32, tag="biask")
                nc.scalar.activation(out=biask, in_=psv[:, 0:1],
                                     func=AF.Identity, scale=-1.0)
                P[p]["biask"] = biask

            # ---------------- Phase C: solve ---------------------------
            for p in range(NP):
                slab = slabp.tile([128, B], BF16, tag="slab")
                nc.sync.dma_start(out=slab, in_=d_xs.ap()[p])
                zpair = zp.tile([128, B], BF16, tag="zpair")
                wt, biask = P[p]["wt"], P[p]["biask"]
                for j in range(NB):
                    psz = ps_big.tile([128, 512], F32, tag="big")
                    nc.tensor.matmul(psz, wt,
                                     slab[:, 512 * j:512 * (j + 1)],
                                     start=True, stop=True)
                    dst = zpair[:, 512 * j:512 * (j + 1)]
                    if j % 2 == 0:
                        nc.scalar.activation(out=dst, in_=psz,
                                             func=AF.Identity, bias=biask)
                    else:
                        nc.vector.tensor_scalar_add(dst, psz, biask)
                nc.sync.dma_start(out=d_out.ap()[p], in_=zpair)

    nc.finalize()
    return nc


def _make_consts():
    i64 = np.eye(64, dtype=np.float32)
    cm64 = np.triu(np.ones((64, 64), np.float32), 1) + 0.5 * i64
    z = np.zeros((64, 64), np.float32)
    cmask = np.block([[cm64, z], [z, cm64]])
    o64 = np.ones((64, 64), np.float32)
    blkm = np.block([[o64, z], [z, o64]])
    i128 = np.eye(128, dtype=np.float32)
    epsi = EPS * i128
    c15 = 1.5 * i128
    c15e = (1.5 - 0.5 * EPS) * i128
    return np.concatenate([cmask, blkm, epsi, c15, c15e], axis=1)


def _prep_inputs(x):
    """x: [B, D, K] fp32 -> per-core input dicts."""
    x = np.asarray(x, dtype=np.float32)
    consts = _make_consts()
    id16 = np.eye(128, dtype=np.float16)
    xb16 = x.astype(ml_dtypes.bfloat16)

    # xs: [K//2 pairs, 128, B]: rows c*64+d = x[:, d, 2p+c]
    xs_full = np.ascontiguousarray(
        xb16.transpose(2, 1, 0).reshape(K // 2, 128, B))

    # xb: [K//2, 128, NCH*130]: [i, 130j + c*64+d] = x[128j+i, d, 2p+c],
    # col 130j+128 = 1, col 130j+129 = 0
    a = xb16.reshape(NCH, 128, D, K // 2, 2)      # j, i, d, p, c
    a = a.transpose(3, 1, 0, 4, 2)                 # p, i, j, c, d
    xb_full = np.zeros((K // 2, 128, NCH, 130), dtype=ml_dtypes.bfloat16)
    xb_full[:, :, :, 0:128] = a.reshape(K // 2, 128, NCH, 128)
    xb_full[:, :, :, 128] = np.float32(1.0)
    xb_full = xb_full.reshape(K // 2, 128, NCH * 130)

    in_maps = []
    for c in range(NCORES):
        ps = slice(c * NP, (c + 1) * NP)
        in_maps.append({"xb": np.ascontiguousarray(xb_full[ps]),
                        "xs": np.ascontiguousarray(xs_full[ps]),
                        "cs": consts, "id16": id16})
    return in_maps


def _run(x, trace=False):
    if "nc" not in _cache:
        _cache["nc"] = _build_nc()
    nc = _cache["nc"]
    in_maps = _prep_inputs(x)
    res = run_bass_kernel_spmd(nc, in_maps, core_ids=list(range(NCORES)),
                               trace=trace)
    out = np.empty((B, D, K), dtype=np.float32)
    for c in range(NCORES):
        ks = slice(c * KL, (c + 1) * KL)
        zo = np.asarray(res.results[c]["out"], dtype=np.float32)
        out[:, :, ks] = zo.reshape(NP, 2, 64, B).transpose(3, 2, 0, 1) \
                          .reshape(B, D, KL)
    return out, res


def kernel(x):
    out, _ = _run(x, trace=False)
    return out


# revision 8
# speedup vs baseline: 3.2491x; 1.1376x over previous
"""ClusterNorm1d TRN2 kernel (pair-batched, low-precision, phase-pipelined).

Math (per cluster k): mu = mean_b x[b,:,k]; cov = centered second moment;
L = chol(cov + eps I); Z = L^-1 (x - mu).

Per core: 32 clusters processed as 16 PAIRS (two 64-dim clusters batched
block-diagonally into 128-wide PE ops).

  - stats: per pair, 32 accumulating bf16 matmuls G += U_j^T [U_j | 1]
    ([128,129] out in fp32 PSUM).  Diag 64x64 blocks of G[:, :128] are the
    two clusters' Grams; col 128 is the per-column sum s (mean * B).
  - cov -> W = L^-1 via Newton on the Cholesky manifold, all in fp16
    (1 cyc/row on PE vs 4 for fp32; iteration is self-correcting):
      P = W A W^T;  I + C^T = 1.5 I - CM o P;  W <- (I+C) W
    keeping both W and W^T via two matmuls per step (no transposes).
  - solve: Z = W x - (W mu) 1^T as bf16 matmuls, mean applied as
    per-partition bias during the PSUM -> SBUF copy; output stored bf16.
Inputs shipped bf16 twice (b-major for stats, d-major for solve), output
bf16: ~51 MB HBM traffic per core vs 85 MB for the f32 layout.
"""
import sys
sys.path.insert(0, "/opt/trn_rl_repo")

import numpy as np
import ml_dtypes

import concourse.bass as bass
from concourse import bacc
import concourse.mybir as mybir
import concourse.tile as tile
from concourse.bass_utils import run_bass_kernel_spmd

B, D, K, NCORES = 4096, 64, 256, 8
KL = K // NCORES          # clusters per core (32)
NP = KL // 2              # cluster pairs per core (16)
NCH = B // 128            # stats chunks (32)
NB = B // 512             # solve chunks per pair (8)
EPS = 1e-4
NIT = 3                   # newton steps incl analytic it0
AF = mybir.ActivationFunctionType
F32 = mybir.dt.float32
F16 = mybir.dt.float16
BF16 = mybir.dt.bfloat16

_cache = {}


def _build_nc():
    nc = bacc.Bacc("TRN2", target_bir_lowering=False, debug=False,
                   num_devices=NCORES)
    d_xb = nc.dram_tensor("xb", [NP, 128, NCH * 130], BF16,
                          kind="ExternalInput")
    d_xs = nc.dram_tensor("xs", [NP, 128, B], BF16, kind="ExternalInput")
    d_cs = nc.dram_tensor("cs", [128, 5 * 128], F32, kind="ExternalInput")
    d_id = nc.dram_tensor("id16", [128, 128], F16, kind="ExternalInput")
    d_out = nc.dram_tensor("out", [NP, 128, B], BF16, kind="ExternalOutput")

    inv_b = 1.0 / B
    a_cov = 1.0 / (B - 1)
    sq_bcov = float(np.sqrt(1.0 / (B * (B - 1.0))))

    with tile.TileContext(nc) as tc:
        with tc.tile_pool(name="consts", bufs=1) as consts, \
             tc.tile_pool(name="ubp", bufs=3) as ubp, \
             tc.tile_pool(name="slabp", bufs=8) as slabp, \
             tc.tile_pool(name="zp", bufs=3) as zp, \
             tc.tile_pool(name="smf", bufs=3) as smf, \
             tc.tile_pool(name="s16", bufs=3) as s16, \
             tc.tile_pool(name="wp", bufs=2 * NP + 4) as wp, \
             tc.tile_pool(name="pp", bufs=NP) as pp, \
             tc.tile_pool(name="ps_big", bufs=2, space="PSUM") as ps_big, \
             tc.tile_pool(name="ps_sm", bufs=4, space="PSUM") as ps_sm, \
             tc.tile_pool(name="ps_row", bufs=1, space="PSUM") as ps_row:

            cs = consts.tile([128, 5 * 128], F32)
            nc.sync.dma_start(out=cs, in_=d_cs.ap())
            cmask = cs[:, 0:128]          # blkdiag(triu(64,1) + 0.5 I)
            blkm = cs[:, 128:256]         # blkdiag(ones 64x64)
            epsi = cs[:, 256:384]         # EPS * I
            c15 = cs[:, 384:512]          # 1.5 * I
            c15e = cs[:, 512:640]         # (1.5 - 0.5*EPS) * I
            id16 = consts.tile([128, 128], F16)
            nc.sync.dma_start(out=id16, in_=d_id.ap())

            P = [dict() for _ in range(NP)]

            # ---------------- Phase A: stats + cov prep ----------------
            def emit_prep(p):
                psg = P[p]["psg"]
                srow = s16.tile([128, 1], F16, tag="srow")
                nc.scalar.activation(out=srow, in_=psg[:, 128:129],
                                     func=AF.Identity, scale=sq_bcov)
                mur = pp.tile([128, 1], BF16, tag="mur")
                nc.scalar.activation(out=mur, in_=psg[:, 128:129],
                                     func=AF.Identity, scale=inv_b)
                pst = ps_row.tile([1, 128], F16, tag="pst")
                nc.tensor.transpose(pst, srow, id16)
                z2 = s16.tile([1, 128], F16, tag="z2")
                nc.scalar.copy(z2, pst)
                pso = ps_sm.tile([128, 128], F32, tag="sm")
                nc.tensor.matmul(pso, z2, z2, start=True, stop=True)
                t1 = smf.tile([128, 128], F32, tag="t1")
                nc.vector.tensor_scalar_mul(t1, psg[:, 0:128], a_cov)
                t3 = smf.tile([128, 128], F32, tag="t3")
                nc.vector.tensor_sub(t3, t1, pso)
                u1 = smf.tile([128, 128], F32, tag="u1")
                nc.vector.tensor_mul(u1, cmask, t3)
                w1t = wp.tile([128, 128], F16, tag="wt")
                nc.vector.tensor_sub(w1t, c15e, u1)
                am = smf.tile([128, 128], F32, tag="am")
                nc.vector.tensor_mul(am, blkm, t3)
                amat = pp.tile([128, 128], F16, tag="amat")
                nc.vector.tensor_add(amat, am, epsi)
                P[p].update(wt=w1t, amat=amat, mur=mur)

            for p in range(NP):
                ub = ubp.tile([128, NCH * 130], BF16, tag="ub")
                nc.sync.dma_start(out=ub, in_=d_xb.ap()[p])
                psg = ps_big.tile([128, 512], F32, tag="big")
                for j in range(NCH):
                    nc.tensor.matmul(psg[:, 0:129],
                                     ub[:, 130 * j:130 * j + 128],
                                     ub[:, 130 * j:130 * j + 129],
                                     start=(j == 0), stop=(j == NCH - 1))
                P[p]["psg"] = psg
                if p >= 1:
                    emit_prep(p - 1)
            emit_prep(NP - 1)

            # ---------------- Phase B: Newton (step-interleaved) -------
            # w1 = (w1t)^T via matmul against identity
            for p in range(NP):
                psw1 = ps_sm.tile([128, 128], F32, tag="sm")
                nc.tensor.matmul(psw1, P[p]["wt"], id16, start=True,
                                 stop=True)
                w = wp.tile([128, 128], F16, tag="w")
                nc.scalar.copy(w, psw1)
                P[p]["w"] = w

            for it in range(1, NIT):
                last = (it == NIT - 1)
                for p in range(NP):
                    psh = ps_sm.tile([128, 128], F32, tag="sm")
                    nc.tensor.matmul(psh, P[p]["amat"], P[p]["wt"],
                                     start=True, stop=True)
                    h = s16.tile([128, 128], F16, tag="h", bufs=NP + 2)
                    nc.scalar.copy(h, psh)
                    P[p]["h"] = h
                for p in range(NP):
                    psp = ps_sm.tile([128, 128], F32, tag="sm")
                    nc.tensor.matmul(psp, P[p]["wt"], P[p]["h"],
                                     start=True, stop=True)
                    u1 = smf.tile([128, 128], F32, tag="u1")
                    nc.vector.tensor_mul(u1, cmask, psp)
                    ctI = s16.tile([128, 128], F16, tag="ct", bufs=NP + 2)
                    nc.vector.tensor_sub(ctI, c15, u1)
                    P[p]["ctI"] = ctI
                for p in range(NP):
                    pswt = ps_sm.tile([128, 128], F32, tag="sm")
                    nc.tensor.matmul(pswt, P[p]["w"], P[p]["ctI"],
                                     start=True, stop=True)
                    if last:
                        wtn = wp.tile([128, 128], BF16, tag="wtb")
                        nc.scalar.copy(wtn, pswt)
                        P[p]["wt"] = wtn
                    else:
                        wtn = wp.tile([128, 128], F16, tag="wt")
                        nc.scalar.copy(wtn, pswt)
                        P[p]["wt"] = wtn
                        psw = ps_sm.tile([128, 128], F32, tag="sm")
                        nc.tensor.matmul(psw, P[p]["ctI"], P[p]["w"],
                                         start=True, stop=True)
                        wn = wp.tile([128, 128], F16, tag="w")
                        nc.scalar.copy(wn, psw)
                        P[p]["w"] = wn

            # bias vector v = W mu (bf16 matmul, N=1) in its own row so
            # psv(p) never waits on the just-written wt copy
            for p in range(NP):
                psv = ps_row.tile([128, 2], F32, tag="psv")
                nc.tensor.matmul(psv[:, 0:1], P[p]["wt"], P[p]["mur"],
                                 start=True, stop=True)
                biask = pp.tile([128, 1], F32, tag="biask")
                nc.scalar.activation(out=biask, in_=psv[:, 0:1],
                                     func=AF.Identity, scale=-1.0)
                P[p]["biask"] = biask

            # ---------------- Phase C: solve ---------------------------
            for p in range(NP):
                slab = slabp.tile([128, B], BF16, tag="slab")
                nc.sync.dma_start(out=slab, in_=d_xs.ap()[p])
                zpair = zp.tile([128, B], BF16, tag="zpair")
                wt, biask = P[p]["wt"], P[p]["biask"]
                for j in range(NB):
                    psz = ps_big.tile([128, 512], F32, tag="big")
                    nc.tensor.matmul(psz, wt,
                                     slab[:, 512 * j:512 * (j + 1)],
                                     start=True, stop=True)
                    dst = zpair[:, 512 * j:512 * (j + 1)]
                    if j % 2 == 0:
                        nc.scalar.activation(out=dst, in_=psz,
                                             func=AF.Identity, bias=biask)
                    else:
                        nc.vector.tensor_scalar_add(dst, psz, biask)
                nc.sync.dma_start(out=d_out.ap()[p], in_=zpair)

    nc.finalize()
    return nc


def _make_consts():
    i64 = np.eye(64, dtype=np.float32)
    cm64 = np.triu(np.ones((64, 64), np.float32), 1) + 0.5 * i64
    z = np.zeros((64, 64), np.float32)
    cmask = np.block([[cm64, z], [z, cm64]])
    o64 = np.ones((64, 64), np.float32)
    blkm = np.block([[o64, z], [z, o64]])
    i128 = np.eye(128, dtype=np.float32)
    epsi = EPS * i128
    c15 = 1.5 * i128
    c15e = (1.5 - 0.5 * EPS) * i128
    return np.concatenate([cmask, blkm, epsi, c15, c15e], axis=1)


def _prep_inputs(x):
    """x: [B, D, K] fp32 -> per-core input dicts."""
    x = np.asarray(x, dtype=np.float32)
    consts = _make_consts()
    id16 = np.eye(128, dtype=np.float16)
    xb16 = x.astype(ml_dtypes.bfloat16)

    # xs: [K//2 pairs, 128, B]: rows c*64+d = x[:, d, 2p+c]
    xs_full = np.ascontiguousarray(
        xb16.transpose(2, 1, 0).reshape(K // 2, 128, B))

    # xb: [K//2, 128, NCH*130]: [i, 130j + c*64+d] = x[128j+i, d, 2p+c],
    # col 130j+128 = 1, col 130j+129 = 0
    a = xb16.reshape(NCH, 128, D, K // 2, 2)      # j, i, d, p, c
    a = a.transpose(3, 1, 0, 4, 2)                 # p, i, j, c, d
    xb_full = np.zeros((K // 2, 128, NCH, 130), dtype=ml_dtypes.bfloat16)
    xb_full[:, :, :, 0:128] = a.reshape(K // 2, 128, NCH, 128)
    xb_full[:, :, :, 128] = np.float32(1.0)
    xb_full = xb_full.reshape(K // 2, 128, NCH * 130)

    in_maps = []
    for c in range(NCORES):
        ps = slice(c * NP, (c + 1) * NP)
        in_maps.append({"xb": np.ascontiguousarray(xb_full[ps]),
                        "xs": np.ascontiguousarray(xs_full[ps]),
                        "cs": consts, "id16": id16})
    return in_maps


def _run(x, trace=False):
    if "nc" not in _cache:
        _cache["nc"] = _build_nc()
    nc = _cache["nc"]
    in_maps = _prep_inputs(x)
    res = run_bass_kernel_spmd(nc, in_maps, core_ids=list(range(NCORES)),
                               trace=trace)
    out = np.empty((B, D, K), dtype=np.float32)
    for c in range(NCORES):
        ks = slice(c * KL, (c + 1) * KL)
        zo = np.asarray(res.results[c]["out"], dtype=np.float32)
        out[:, :, ks] = zo.reshape(NP, 2, 64, B).transpose(3, 2, 0, 1) \
                          .reshape(B, D, KL)
    return out, res


def kernel(x):
    out, _ = _run(x, trace=False)
    return out


# revision 10
# speedup vs baseline: 4.0459x; 1.2452x over previous
"""ClusterNorm1d TRN2 kernel (pair-batched, low-precision, phase-pipelined).

Math (per cluster k): mu = mean_b x[b,:,k]; cov = centered second moment;
L = chol(cov + eps I); Z = L^-1 (x - mu).

Per core: 32 clusters processed as 16 PAIRS (two 64-dim clusters batched
block-diagonally into 128-wide PE ops).

  - stats: per pair, 32 accumulating bf16 matmuls G += U_j^T [U_j | 1]
    ([128,129] out in fp32 PSUM).  Diag 64x64 blocks of G[:, :128] are the
    two clusters' Grams; col 128 is the per-column sum s (mean * B).
  - cov -> W = L^-1 via Newton on the Cholesky manifold, all in fp16
    (1 cyc/row on PE vs 4 for fp32; iteration is self-correcting):
      P = W A W^T;  I + C^T = 1.5 I - CM o P;  W <- (I+C) W
    keeping both W and W^T via two matmuls per step (no transposes).
  - solve: Z = W x - (W mu) 1^T as bf16 matmuls, mean applied as
    per-partition bias during the PSUM -> SBUF copy; output stored bf16.
Inputs shipped bf16 twice (b-major for stats, d-major for solve), output
bf16: ~51 MB HBM traffic per core vs 85 MB for the f32 layout.
"""
import sys
sys.path.insert(0, "/opt/trn_rl_repo")

import numpy as np
import ml_dtypes

import concourse.bass as bass
from concourse import bacc
import concourse.mybir as mybir
import concourse.tile as tile
from concourse.bass_utils import run_bass_kernel_spmd

B, D, K, NCORES = 4096, 64, 256, 8
KL = K // NCORES          # clusters per core (32)
NP = KL // 2              # cluster pairs per core (16)
NCH = B // 128            # stats chunks (32)
NB = B // 512             # solve chunks per pair (8)
EPS = 1e-4
NIT = 3                   # newton steps incl analytic it0
AF = mybir.ActivationFunctionType
F32 = mybir.dt.float32
F16 = mybir.dt.float16
BF16 = mybir.dt.bfloat16
F8 = mybir.dt.float8e4

_cache = {}


def _build_nc():
    nc = bacc.Bacc("TRN2", target_bir_lowering=False, debug=False,
                   num_devices=NCORES)
    d_xb = nc.dram_tensor("xb", [NP, 128, NCH * 130], F8,
                          kind="ExternalInput")
    d_xs = nc.dram_tensor("xs", [NP, 128, B], BF16, kind="ExternalInput")
    d_cs = nc.dram_tensor("cs", [128, 5 * 128], F32, kind="ExternalInput")
    d_id = nc.dram_tensor("id16", [128, 128], F16, kind="ExternalInput")
    d_out = nc.dram_tensor("out", [NP, 128, B], BF16, kind="ExternalOutput")

    inv_b = 1.0 / B
    a_cov = 1.0 / (B - 1)
    sq_bcov = float(np.sqrt(1.0 / (B * (B - 1.0))))

    with tile.TileContext(nc) as tc:
        with tc.tile_pool(name="consts", bufs=1) as consts, \
             tc.tile_pool(name="ubp", bufs=3) as ubp, \
             tc.tile_pool(name="slabp", bufs=12) as slabp, \
             tc.tile_pool(name="zp", bufs=3) as zp, \
             tc.tile_pool(name="smf", bufs=3) as smf, \
             tc.tile_pool(name="s16", bufs=3) as s16, \
             tc.tile_pool(name="wp", bufs=2 * NP + 4) as wp, \
             tc.tile_pool(name="pp", bufs=NP) as pp, \
             tc.tile_pool(name="ps_big", bufs=2, space="PSUM") as ps_big, \
             tc.tile_pool(name="ps_sm", bufs=4, space="PSUM") as ps_sm, \
             tc.tile_pool(name="ps_row", bufs=1, space="PSUM") as ps_row:

            cs = consts.tile([128, 5 * 128], F32)
            nc.sync.dma_start(out=cs, in_=d_cs.ap())
            cmask = cs[:, 0:128]          # blkdiag(triu(64,1) + 0.5 I)
            blkm = cs[:, 128:256]         # blkdiag(ones 64x64)
            epsi = cs[:, 256:384]         # EPS * I
            c15 = cs[:, 384:512]          # 1.5 * I
            c15e = cs[:, 512:640]         # (1.5 - 0.5*EPS) * I
            id16 = consts.tile([128, 128], F16)
            nc.sync.dma_start(out=id16, in_=d_id.ap())

            P = [dict() for _ in range(NP)]

            # ---------------- Phase A: stats + cov prep ----------------
            def emit_prep(p):
                psg = P[p]["psg"]
                srow = s16.tile([128, 1], F16, tag="srow")
                nc.scalar.activation(out=srow, in_=psg[:, 128:129],
                                     func=AF.Identity, scale=sq_bcov)
                mur = pp.tile([128, 1], BF16, tag="mur")
                nc.scalar.activation(out=mur, in_=psg[:, 128:129],
                                     func=AF.Identity, scale=inv_b)
                pst = ps_row.tile([1, 128], F16, tag="pst")
                nc.tensor.transpose(pst, srow, id16)
                z2 = s16.tile([1, 128], F16, tag="z2")
                nc.scalar.copy(z2, pst)
                pso = ps_sm.tile([128, 128], F32, tag="sm")
                nc.tensor.matmul(pso, z2, z2, start=True, stop=True)
                t1 = smf.tile([128, 128], F32, tag="t1")
                nc.vector.tensor_scalar_mul(t1, psg[:, 0:128], a_cov)
                t3 = smf.tile([128, 128], F32, tag="t3")
                nc.vector.tensor_sub(t3, t1, pso)
                u1 = smf.tile([128, 128], F32, tag="u1")
                nc.vector.tensor_mul(u1, cmask, t3)
                w1t = wp.tile([128, 128], F16, tag="wt")
                nc.vector.tensor_sub(w1t, c15e, u1)
                am = smf.tile([128, 128], F32, tag="am")
                nc.vector.tensor_mul(am, blkm, t3)
                amat = pp.tile([128, 128], F16, tag="amat")
                nc.vector.tensor_add(amat, am, epsi)
                P[p].update(wt=w1t, amat=amat, mur=mur)

            for p in range(NP):
                ub = ubp.tile([128, NCH * 130], F8, tag="ub")
                nc.sync.dma_start(out=ub, in_=d_xb.ap()[p])
                psg = ps_big.tile([128, 512], F32, tag="big")
                for j in range(NCH):
                    nc.tensor.matmul(psg[:, 0:129],
                                     ub[:, 130 * j:130 * j + 128],
                                     ub[:, 130 * j:130 * j + 129],
                                     start=(j == 0), stop=(j == NCH - 1))
                P[p]["psg"] = psg
                if p >= 1:
                    emit_prep(p - 1)
            emit_prep(NP - 1)

            # ---------------- Phase B: Newton (step-interleaved) -------
            # w1 = (w1t)^T via matmul against identity
            for p in range(NP):
                psw1 = ps_sm.tile([128, 128], F32, tag="sm")
                nc.tensor.matmul(psw1, P[p]["wt"], id16, start=True,
                                 stop=True)
                w = wp.tile([128, 128], F16, tag="w")
                nc.scalar.copy(w, psw1)
                P[p]["w"] = w

            for it in range(1, NIT):
                last = (it == NIT - 1)
                for p in range(NP):
                    psh = ps_sm.tile([128, 128], F32, tag="sm")
                    nc.tensor.matmul(psh, P[p]["amat"], P[p]["wt"],
                                     start=True, stop=True)
                    h = s16.tile([128, 128], F16, tag="h", bufs=NP + 2)
                    nc.scalar.copy(h, psh)
                    P[p]["h"] = h
                for p in range(NP):
                    psp = ps_sm.tile([128, 128], F32, tag="sm")
                    nc.tensor.matmul(psp, P[p]["wt"], P[p]["h"],
                                     start=True, stop=True)
                    u1 = smf.tile([128, 128], F32, tag="u1")
                    nc.vector.tensor_mul(u1, cmask, psp)
                    ctI = s16.tile([128, 128], F16, tag="ct", bufs=NP + 2)
                    nc.vector.tensor_sub(ctI, c15, u1)
                    P[p]["ctI"] = ctI
                for p in range(NP):
                    pswt = ps_sm.tile([128, 128], F32, tag="sm")
                    nc.tensor.matmul(pswt, P[p]["w"], P[p]["ctI"],
                                     start=True, stop=True)
                    if last:
                        wtn = wp.tile([128, 128], BF16, tag="wtb")
                        nc.scalar.copy(wtn, pswt)
                        P[p]["wt"] = wtn
                    else:
                        wtn = wp.tile([128, 128], F16, tag="wt")
                        nc.scalar.copy(wtn, pswt)
                        P[p]["wt"] = wtn
                        psw = ps_sm.tile([128, 128], F32, tag="sm")
                        nc.tensor.matmul(psw, P[p]["ctI"], P[p]["w"],
                                         start=True, stop=True)
                        wn = wp.tile([128, 128], F16, tag="w")
                        nc.scalar.copy(wn, psw)
                        P[p]["w"] = wn

            # bias vector v = W mu (bf16 matmul, N=1) in its own row so
            # psv(p) never waits on the just-written wt copy
            for p in range(NP):
                psv = ps_row.tile([128, 2], F32, tag="psv")
                nc.tensor.matmul(psv[:, 0:1], P[p]["wt"], P[p]["mur"],
                                 start=True, stop=True)
                biask = pp.tile([128, 1], F32, tag="biask")
                nc.scalar.activation(out=biask, in_=psv[:, 0:1],
                                     func=AF.Identity, scale=-1.0)
                P[p]["biask"] = biask

            # ---------------- Phase C: solve ---------------------------
            for p in range(NP):
                slab = slabp.tile([128, B], BF16, tag="slab")
                nc.sync.dma_start(out=slab, in_=d_xs.ap()[p])
                zpair = zp.tile([128, B], BF16, tag="zpair")
                wt, biask = P[p]["wt"], P[p]["biask"]
                for j in range(NB):
                    if j % 2 == 0:
                        psz = ps_big.tile([128, 512], F32, tag="big")
                    else:
                        psz = ps_sm.tile([128, 512], F32, tag="sm")
                    nc.tensor.matmul(psz, wt,
                                     slab[:, 512 * j:512 * (j + 1)],
                                     start=True, stop=True)
                    dst = zpair[:, 512 * j:512 * (j + 1)]
                    if j % 2 == 0:
                        nc.scalar.activation(out=dst, in_=psz,
                                             func=AF.Identity, bias=biask)
                    else:
                        nc.vector.tensor_scalar_add(dst, psz, biask)
                    if j == NB // 2 - 1:
                        nc.sync.dma_start(out=d_out.ap()[p][:, 0:B // 2],
                                          in_=zpair[:, 0:B // 2])
                nc.sync.dma_start(out=d_out.ap()[p][:, B // 2:B],
                                  in_=zpair[:, B // 2:B])

    nc.finalize()
    return nc


def _make_consts():
    i64 = np.eye(64, dtype=np.float32)
    cm64 = np.triu(np.ones((64, 64), np.float32), 1) + 0.5 * i64
    z = np.zeros((64, 64), np.float32)
    cmask = np.block([[cm64, z], [z, cm64]])
    o64 = np.ones((64, 64), np.float32)
    blkm = np.block([[o64, z], [z, o64]])
    i128 = np.eye(128, dtype=np.float32)
    epsi = EPS * i128
    c15 = 1.5 * i128
    c15e = (1.5 - 0.5 * EPS) * i128
    return np.concatenate([cmask, blkm, epsi, c15, c15e], axis=1)


def _prep_inputs(x):
    """x: [B, D, K] fp32 -> per-core input dicts."""
    x = np.asarray(x, dtype=np.float32)
    consts = _make_consts()
    id16 = np.eye(128, dtype=np.float16)
    # xs: [K//2 pairs, 128, B]: rows c*64+d = x[:, d, 2p+c]
    xs_full = np.ascontiguousarray(
        x.transpose(2, 1, 0).reshape(K // 2, 128, B).astype(
            ml_dtypes.bfloat16))

    # xb (fp8 e4m3): [K//2, 128, NCH*130]: [i, 130j + c*64+d] =
    # x[128j+i, d, 2p+c], col 130j+128 = 1, col 130j+129 = 0
    a = x.reshape(NCH, 128, D, K // 2, 2)          # j, i, d, p, c
    a = a.transpose(3, 1, 0, 4, 2)                 # p, i, j, c, d
    xb_full = np.zeros((K // 2, 128, NCH, 130), dtype=ml_dtypes.float8_e4m3)
    xb_full[:, :, :, 0:128] = a.reshape(K // 2, 128, NCH, 128).astype(
        ml_dtypes.float8_e4m3)
    xb_full[:, :, :, 128] = np.float32(1.0)
    xb_full = xb_full.reshape(K // 2, 128, NCH * 130)

    in_maps = []
    for c in range(NCORES):
        ps = slice(c * NP, (c + 1) * NP)
        in_maps.append({"xb": np.ascontiguousarray(xb_full[ps]),
                        "xs": np.ascontiguousarray(xs_full[ps]),
                        "cs": consts, "id16": id16})
    return in_maps


def _run(x, trace=False):
    if "nc" not in _cache:
        _cache["nc"] = _build_nc()
    nc = _cache["nc"]
    in_maps = _prep_inputs(x)
    res = run_bass_kernel_spmd(nc, in_maps, core_ids=list(range(NCORES)),
                               trace=trace)
    out = np.empty((B, D, K), dtype=np.float32)
    for c in range(NCORES):
        ks = slice(c * KL, (c + 1) * KL)
        zo = np.asarray(res.results[c]["out"], dtype=np.float32)
        out[:, :, ks] = zo.reshape(NP, 2, 64, B).transpose(3, 2, 0, 1) \
                          .reshape(B, D, KL)
    return out, res


def kernel(x):
    out, _ = _run(x, trace=False)
    return out


# revision 11
# speedup vs baseline: 4.6676x; 1.1537x over previous
"""ClusterNorm1d TRN2 kernel (pair-batched, low-precision, phase-pipelined).

Math (per cluster k): mu = mean_b x[b,:,k]; cov = centered second moment;
L = chol(cov + eps I); Z = L^-1 (x - mu).

Per core: 32 clusters processed as 16 PAIRS (two 64-dim clusters batched
block-diagonally into 128-wide PE ops).

  - stats: per pair, 32 accumulating bf16 matmuls G += U_j^T [U_j | 1]
    ([128,129] out in fp32 PSUM).  Diag 64x64 blocks of G[:, :128] are the
    two clusters' Grams; col 128 is the per-column sum s (mean * B).
  - cov -> W = L^-1 via Newton on the Cholesky manifold, all in fp16
    (1 cyc/row on PE vs 4 for fp32; iteration is self-correcting):
      P = W A W^T;  I + C^T = 1.5 I - CM o P;  W <- (I+C) W
    keeping both W and W^T via two matmuls per step (no transposes).
  - solve: Z = W x - (W mu) 1^T as bf16 matmuls, mean applied as
    per-partition bias during the PSUM -> SBUF copy; output stored bf16.
Inputs shipped bf16 twice (b-major for stats, d-major for solve), output
bf16: ~51 MB HBM traffic per core vs 85 MB for the f32 layout.
"""
import sys
sys.path.insert(0, "/opt/trn_rl_repo")

import numpy as np
import ml_dtypes

import concourse.bass as bass
from concourse import bacc
import concourse.mybir as mybir
import concourse.tile as tile
from concourse.bass_utils import run_bass_kernel_spmd

B, D, K, NCORES = 4096, 64, 256, 8
KL = K // NCORES          # clusters per core (32)
NP = KL // 2              # cluster pairs per core (16)
NCH = B // 128            # stats chunks (32)
NB = B // 512             # solve chunks per pair (8)
EPS = 1e-4
NIT = 3                   # newton steps incl analytic it0
AF = mybir.ActivationFunctionType
F32 = mybir.dt.float32
F16 = mybir.dt.float16
BF16 = mybir.dt.bfloat16
F8 = mybir.dt.float8e4

_cache = {}


def _build_nc():
    nc = bacc.Bacc("TRN2", target_bir_lowering=False, debug=False,
                   num_devices=NCORES)
    d_xb = nc.dram_tensor("xb", [NP, 128, NCH * 130], F8,
                          kind="ExternalInput")
    d_xs = nc.dram_tensor("xs", [NP, 128, B], BF16, kind="ExternalInput")
    d_cs = nc.dram_tensor("cs", [128, 5 * 128], F32, kind="ExternalInput")
    d_id = nc.dram_tensor("id16", [128, 128], F16, kind="ExternalInput")
    d_out = nc.dram_tensor("out", [NP, 128, B], BF16, kind="ExternalOutput")

    inv_b = 1.0 / B
    a_cov = 1.0 / (B - 1)
    sq_bcov = float(np.sqrt(1.0 / (B * (B - 1.0))))

    with tile.TileContext(nc) as tc:
        with tc.tile_pool(name="consts", bufs=1) as consts, \
             tc.tile_pool(name="ubp", bufs=3) as ubp, \
             tc.tile_pool(name="slabp", bufs=12) as slabp, \
             tc.tile_pool(name="zp", bufs=3) as zp, \
             tc.tile_pool(name="smf", bufs=3) as smf, \
             tc.tile_pool(name="s16", bufs=3) as s16, \
             tc.tile_pool(name="wp", bufs=2 * NP + 4) as wp, \
             tc.tile_pool(name="pp", bufs=NP) as pp, \
             tc.tile_pool(name="ps_big", bufs=2, space="PSUM") as ps_big, \
             tc.tile_pool(name="ps_sm", bufs=4, space="PSUM") as ps_sm, \
             tc.tile_pool(name="ps_row", bufs=1, space="PSUM") as ps_row:

            cs = consts.tile([128, 5 * 128], F32)
            nc.sync.dma_start(out=cs, in_=d_cs.ap())
            cmask = cs[:, 0:128]          # blkdiag(triu(64,1) + 0.5 I)
            blkm = cs[:, 128:256]         # blkdiag(ones 64x64)
            epsi = cs[:, 256:384]         # EPS * I
            c15 = cs[:, 384:512]          # 1.5 * I
            c15e = cs[:, 512:640]         # (1.5 - 0.5*EPS) * I
            id16 = consts.tile([128, 128], F16)
            nc.sync.dma_start(out=id16, in_=d_id.ap())

            P = [dict() for _ in range(NP)]

            # ---------------- Phase A: stats + cov prep ----------------
            def emit_prep(p):
                psg = P[p]["psg"]
                srow = s16.tile([128, 1], F16, tag="srow")
                nc.scalar.activation(out=srow, in_=psg[:, 128:129],
                                     func=AF.Identity, scale=sq_bcov)
                mur = pp.tile([128, 1], BF16, tag="mur")
                nc.scalar.activation(out=mur, in_=psg[:, 128:129],
                                     func=AF.Identity, scale=inv_b)
                pst = ps_row.tile([1, 128], F16, tag="pst")
                nc.tensor.transpose(pst, srow, id16)
                z2 = s16.tile([1, 128], F16, tag="z2")
                nc.scalar.copy(z2, pst)
                pso = ps_sm.tile([128, 128], F32, tag="sm")
                nc.tensor.matmul(pso, z2, z2, start=True, stop=True)
                t1 = smf.tile([128, 128], F32, tag="t1")
                nc.vector.tensor_scalar_mul(t1, psg[:, 0:128], a_cov)
                t3 = smf.tile([128, 128], F32, tag="t3")
                nc.vector.tensor_sub(t3, t1, pso)
                u1 = smf.tile([128, 128], F32, tag="u1")
                nc.vector.tensor_mul(u1, cmask, t3)
                w1t = wp.tile([128, 128], F16, tag="wt")
                nc.vector.tensor_sub(w1t, c15e, u1)
                am = smf.tile([128, 128], F32, tag="am")
                nc.vector.tensor_mul(am, blkm, t3)
                amat = pp.tile([128, 128], F16, tag="amat")
                nc.vector.tensor_add(amat, am, epsi)
                P[p].update(wt=w1t, amat=amat, mur=mur)

            for p in range(NP):
                ub = ubp.tile([128, NCH * 130], F8, tag="ub")
                nc.sync.dma_start(out=ub, in_=d_xb.ap()[p])
                psg = ps_big.tile([128, 512], F32, tag="big")
                for j in range(NCH):
                    nc.tensor.matmul(psg[:, 0:129],
                                     ub[:, 130 * j:130 * j + 128],
                                     ub[:, 130 * j:130 * j + 129],
                                     start=(j == 0), stop=(j == NCH - 1))
                P[p]["psg"] = psg
                if p >= 1:
                    emit_prep(p - 1)
            emit_prep(NP - 1)

            # ---- Phase B (Newton, step-interleaved within a group) ----
            def emit_newton(group):
                for p in group:
                    psw1 = ps_sm.tile([128, 128], F32, tag="sm")
                    nc.tensor.matmul(psw1, P[p]["wt"], id16, start=True,
                                     stop=True)
                    w = wp.tile([128, 128], F16, tag="w")
                    nc.scalar.copy(w, psw1)
                    P[p]["w"] = w
                for it in range(1, NIT):
                    last = (it == NIT - 1)
                    for p in group:
                        psh = ps_sm.tile([128, 128], F32, tag="sm")
                        nc.tensor.matmul(psh, P[p]["amat"], P[p]["wt"],
                                         start=True, stop=True)
                        h = s16.tile([128, 128], F16, tag="h", bufs=NP + 2)
                        nc.scalar.copy(h, psh)
                        P[p]["h"] = h
                    for p in group:
                        psp = ps_sm.tile([128, 128], F32, tag="sm")
                        nc.tensor.matmul(psp, P[p]["wt"], P[p]["h"],
                                         start=True, stop=True)
                        u1 = smf.tile([128, 128], F32, tag="u1")
                        nc.vector.tensor_mul(u1, cmask, psp)
                        ctI = s16.tile([128, 128], F16, tag="ct",
                                       bufs=NP + 2)
                        nc.vector.tensor_sub(ctI, c15, u1)
                        P[p]["ctI"] = ctI
                    for p in group:
                        pswt = ps_sm.tile([128, 128], F32, tag="sm")
                        nc.tensor.matmul(pswt, P[p]["w"], P[p]["ctI"],
                                         start=True, stop=True)
                        if last:
                            wtn = wp.tile([128, 128], BF16, tag="wtb")
                            nc.scalar.copy(wtn, pswt)
                            P[p]["wt"] = wtn
                        else:
                            wtn = wp.tile([128, 128], F16, tag="wt")
                            nc.scalar.copy(wtn, pswt)
                            P[p]["wt"] = wtn
                            psw = ps_sm.tile([128, 128], F32, tag="sm")
                            nc.tensor.matmul(psw, P[p]["ctI"], P[p]["w"],
                                             start=True, stop=True)
                            wn = wp.tile([128, 128], F16, tag="w")
                            nc.scalar.copy(wn, psw)
                            P[p]["w"] = wn
                for p in group:
                    psv = ps_row.tile([128, 2], F32, tag="psv")
                    nc.tensor.matmul(psv[:, 0:1], P[p]["wt"], P[p]["mur"],
                                     start=True, stop=True)
                    biask = pp.tile([128, 1], F32, tag="biask")
                    nc.scalar.activation(out=biask, in_=psv[:, 0:1],
                                         func=AF.Identity, scale=-1.0)
                    P[p]["biask"] = biask

            # ---- Phase C (solve) ----
            def emit_solve(p):
                slab = slabp.tile([128, B], BF16, tag="slab")
                nc.sync.dma_start(out=slab, in_=d_xs.ap()[p])
                zpair = zp.tile([128, B], BF16, tag="zpair")
                wt, biask = P[p]["wt"], P[p]["biask"]
                for j in range(NB):
                    if j % 2 == 0:
                        psz = ps_big.tile([128, 512], F32, tag="big")
                    else:
                        psz = ps_sm.tile([128, 512], F32, tag="sm")
                    nc.tensor.matmul(psz, wt,
                                     slab[:, 512 * j:512 * (j + 1)],
                                     start=True, stop=True)
                    dst = zpair[:, 512 * j:512 * (j + 1)]
                    if j % 2 == 0:
                        nc.scalar.activation(out=dst, in_=psz,
                                             func=AF.Identity, bias=biask)
                    else:
                        nc.vector.tensor_scalar_add(dst, psz, biask)
                    if j == NB // 2 - 1:
                        nc.sync.dma_start(out=d_out.ap()[p][:, 0:B // 2],
                                          in_=zpair[:, 0:B // 2])
                nc.sync.dma_start(out=d_out.ap()[p][:, B // 2:B],
                                  in_=zpair[:, B // 2:B])

            # interleave: newton for a group of 4 pairs, then its solves,
            # so out-DMA starts early and flows continuously
            GRP = 4
            for g0 in range(0, NP, GRP):
                emit_newton(range(g0, g0 + GRP))
                for p in range(g0, g0 + GRP):
                    emit_solve(p)

    nc.finalize()
    return nc


def _make_consts():
    i64 = np.eye(64, dtype=np.float32)
    cm64 = np.triu(np.ones((64, 64), np.float32), 1) + 0.5 * i64
    z = np.zeros((64, 64), np.float32)
    cmask = np.block([[cm64, z], [z, cm64]])
    o64 = np.ones((64, 64), np.float32)
    blkm = np.block([[o64, z], [z, o64]])
    i128 = np.eye(128, dtype=np.float32)
    epsi = EPS * i128
    c15 = 1.5 * i128
    c15e = (1.5 - 0.5 * EPS) * i128
    return np.concatenate([cmask, blkm, epsi, c15, c15e], axis=1)


def _prep_inputs(x):
    """x: [B, D, K] fp32 -> per-core input dicts."""
    x = np.asarray(x, dtype=np.float32)
    consts = _make_consts()
    id16 = np.eye(128, dtype=np.float16)
    # xs: [K//2 pairs, 128, B]: rows c*64+d = x[:, d, 2p+c]
    xs_full = np.ascontiguousarray(
        x.transpose(2, 1, 0).reshape(K // 2, 128, B).astype(
            ml_dtypes.bfloat16))

    # xb (fp8 e4m3): [K//2, 128, NCH*130]: [i, 130j + c*64+d] =
    # x[128j+i, d, 2p+c], col 130j+128 = 1, col 130j+129 = 0
    a = x.reshape(NCH, 128, D, K // 2, 2)          # j, i, d, p, c
    a = a.transpose(3, 1, 0, 4, 2)                 # p, i, j, c, d
    xb_full = np.zeros((K // 2, 128, NCH, 130), dtype=ml_dtypes.float8_e4m3)
    xb_full[:, :, :, 0:128] = a.reshape(K // 2, 128, NCH, 128).astype(
        ml_dtypes.float8_e4m3)
    xb_full[:, :, :, 128] = np.float32(1.0)
    xb_full = xb_full.reshape(K // 2, 128, NCH * 130)

    in_maps = []
    for c in range(NCORES):
        ps = slice(c * NP, (c + 1) * NP)
        in_maps.append({"xb": np.ascontiguousarray(xb_full[ps]),
                        "xs": np.ascontiguousarray(xs_full[ps]),
                        "cs": consts, "id16": id16})
    return in_maps


def _run(x, trace=False):
    if "nc" not in _cache:
        _cache["nc"] = _build_nc()
    nc = _cache["nc"]
    in_maps = _prep_inputs(x)
    res = run_bass_kernel_spmd(nc, in_maps, core_ids=list(range(NCORES)),
                               trace=trace)
    out = np.empty((B, D, K), dtype=np.float32)
    for c in range(NCORES):
        ks = slice(c * KL, (c + 1) * KL)
        zo = np.asarray(res.results[c]["out"], dtype=np.float32)
        out[:, :, ks] = zo.reshape(NP, 2, 64, B).transpose(3, 2, 0, 1) \
                          .reshape(B, D, KL)
    return out, res


def kernel(x):
    out, _ = _run(x, trace=False)
    return out
